# revision 1
# baseline (speedup 1.0000x reference)
"""Trainium2 Bass kernel for nn_AttnDecoderRNN (B=32,T=20,L=49,F=512,H=1024,V=32000).

Zero-collective design across 8 NeuronCores:
- The attention-LSTM recurrence is fully REPLICATED on every core (tensor-
  parallel splits need a per-step AllGather of h, which dominates cost).
- Only the vocab projection is tensor-parallel: core k owns W_out rows
  [4000k, 4000(k+1)) and emits a (640, 4000) bf16 logit shard; the host
  concatenation is the all-gather. b_out is added host-side (skipped if 0).

Per-core schedule (all layouts feature-on-partition):
- scoresT attention: sps[(l,b), b'] = feats . (Wa^T h), diagonal extracted
  with a one-hot mask + grouped reduce; softmax runs in the (l%4 x b, l//4)
  layout with tiny PE sum/broadcast matmuls; normalize+expand fused into one
  scalar_tensor_tensor.
- gates: one PSUM accumulation (bias matmul + iw + h + ctx parts); the
  LSTM elementwise runs mostly in place on PSUM; a dummy sigmoid after the
  softmax Exp prefetches the ACT table set off the critical chain.
- dec groups (4 steps) with Wh2o/Wc2o streamed from HBM; vocab chunks are
  interleaved into the step loop (W_out streamed) to keep the PE fed
  through the softmax/LSTM dependency-chain windows.
"""
import sys

sys.path.insert(0, "/opt/trn_rl_repo")
import numpy as np
import ml_dtypes

import concourse.bass as bass
import concourse.mybir as mybir
import concourse.tile as tile
from concourse import bacc
from concourse.bass_utils import run_bass_kernel_spmd

B, T, L, F, H, V = 32, 20, 49, 512, 1024, 32000
LP = 64
NC = 8
VS = V // NC      # 4000
CW = 250          # vocab chunk width (16 chunks per group)
BF = mybir.dt.bfloat16
F32 = mybir.dt.float32
NBF = ml_dtypes.bfloat16
NF8 = ml_dtypes.float8_e3m4
WSCALE = 128.0
GPERM = [2, 1, 0, 3]   # gate storage g,f,i,o: bank0={g,f} early, bank1={i,o} late

_BUILT = {}
SECTIONS = []


def _sec(nc, label):
    SECTIONS.append((int(nc.get_next_instruction_name().split('-')[1]), label))


def host_prep(inputs):
    f32 = lambda x: np.asarray(x, np.float32)
    feats = f32(inputs["features"])                    # (B, F, L)
    cap = np.asarray(inputs["captions"])
    emb = np.asarray(inputs["embed_table"])
    fpad = np.zeros((LP, B, F), np.float32)
    fpad[:L] = feats.transpose(2, 0, 1)
    fblk = fpad.reshape(LP * B, F)                     # (2048, 512)
    featsT = np.ascontiguousarray(fblk.T)              # (512, 2048) col l*32+b
    h0 = np.tanh(feats.mean(axis=2) @ f32(inputs["W_init"]).T + f32(inputs["b_init"]))
    h0T = h0.T                                         # (1024, 32)
    h0slot = np.ascontiguousarray(
        h0T.reshape(8, 128, B).transpose(1, 0, 2)).reshape(128, 256)
    e = f32(emb[cap])
    iw = np.concatenate([np.zeros((B, 1, F), np.float32), e[:, :-1]], axis=1)
    iwT = np.ascontiguousarray(iw.transpose(2, 1, 0)).reshape(F, T * B)
    Wih = f32(inputs["W_ih"])
    # permute gate blocks: storage pos p holds original gate GPERM[p]
    def gperm_cols(W):                                 # W (K, 4096) -> permuted
        return np.ascontiguousarray(
            W.reshape(-1, 4, 1024)[:, GPERM].reshape(-1, 4096))
    Whh = gperm_cols(f32(inputs["W_hh"]).T)            # (1024, 4096)
    Wi1 = gperm_cols(Wih[:, :F].T)                     # (512, 4096)
    Wi2 = gperm_cols(Wih[:, F:].T)                     # (512, 4096)
    bg = (f32(inputs["b_ih"]) + f32(inputs["b_hh"])).reshape(4, 8, 128)[GPERM]
    indic32 = np.zeros((32, 1024), np.float32)
    for k in range(32):
        indic32[k, k * 32:(k + 1) * 32] = 1.0
    maskE = np.zeros((128, 512), np.float32)
    for p in range(128):
        maskE[p, np.arange(16) * 32 + (p % 32)] = 1.0
    padT = np.zeros((128, 16), np.float32)
    for p in range(128):
        for m in range(16):
            if m * 4 + p // 32 >= L:
                padT[p, m] = -30000.0
    Pg = np.zeros((128, 32), np.float32)
    for p in range(128):
        Pg[p, p % 32] = 1.0
    Pb = np.ascontiguousarray(Pg.T)
    bdec = (f32(inputs["b_h2o"]) + f32(inputs["b_c2o"])).reshape(4, 128).T
    shared = {
        "featsT": featsT.astype(NBF), "fblk": fblk.astype(NBF),
        "h0slot": h0slot.astype(NBF), "c0": h0slot.astype(np.float32),
        "iwT": iwT.astype(NBF),
        "Wa": f32(inputs["Wa"]).astype(NBF),           # (1024, 512) lhsT
        # gate weights in float8-e3m4 at x128 pre-scale (halves the
        # startup-critical load bytes); compensated by scale=1/128 on the
        # gate activations. x128 lifts the 0.02-scale weights into e3m4's
        # normal range (values +-13 << 31 max).
        "Whh": (Whh * WSCALE).astype(NF8),
        "Wi1": (Wi1 * WSCALE).astype(NF8),
        "Wi2": (Wi2 * WSCALE).astype(NF8),
        "biasLhs": (bg.reshape(32, 128) * WSCALE).astype(NBF),
        "indic32": indic32.astype(NBF),
        "Wh2o": f32(inputs["W_h2o"]).T.astype(NBF),    # (1024, 512)
        "Wc2o": f32(inputs["W_c2o"]).T.astype(NBF),    # (512, 512)
        "bdec": np.ascontiguousarray(bdec),
        "maskE": maskE.astype(NBF), "padT": padT,
        "Pg": Pg, "Pb": Pb,
        "ident": np.eye(128, dtype=np.float32).astype(NBF),
    }
    WoutT = f32(inputs["W_out"]).T                     # (512, 32000)
    in_maps = []
    for k in range(NC):
        m = dict(shared)
        # partition-major chunk layout: row p holds [ci][kt][n] so each
        # chunk DMA is one contiguous (128, 1000-elem) slice (>=512B runs)
        m["Wout"] = np.ascontiguousarray(
            WoutT[:, VS * k:VS * (k + 1)].reshape(4, 128, 16, CW)
            .transpose(1, 2, 0, 3).reshape(128, 64 * CW)).astype(NBF)
        in_maps.append(m)
    return in_maps


def _load_tiled(nc, pool, dram, KT, N, dtype, name):
    """dram (KT*128, N) -> sbuf (128, KT*N), col block kt holds rows kt*128.."""
    t = pool.tile([128, KT * N], dtype, name=name)
    src = dram[:].rearrange("(r p) n -> p r n", p=128)
    dst = t[:].rearrange("p (r n) -> p r n", n=N)
    nc.sync.dma_start(dst, src)
    return t


def build(repeat=1, dbg=False):
    nc = bacc.Bacc("TRN2", target_bir_lowering=False, debug=False, num_devices=NC)
    di = lambda nm, sh, dt: nc.dram_tensor(nm, list(sh), dt, kind="ExternalInput")
    featsT_d = di("featsT", (512, 2048), BF)
    fblk_d = di("fblk", (2048, 512), BF)
    h0_d = di("h0slot", (128, 256), BF)
    c0_d = di("c0", (128, 256), F32)
    iwT_d = di("iwT", (512, 640), BF)
    Wa_d = di("Wa", (1024, 512), BF)
    F8 = mybir.dt.float8e3
    Whh_d = di("Whh", (1024, 4096), F8)
    Wi1_d = di("Wi1", (512, 4096), F8)
    Wi2_d = di("Wi2", (512, 4096), F8)
    biasLhs_d = di("biasLhs", (32, 128), BF)
    indic32_d = di("indic32", (32, 1024), BF)
    Wh2o_d = di("Wh2o", (1024, 512), BF)
    Wc2o_d = di("Wc2o", (512, 512), BF)
    bdec_d = di("bdec", (128, 4), F32)
    Wout_d = di("Wout", (128, 64 * CW), BF)
    maskE_d = di("maskE", (128, 512), BF)
    padT_d = di("padT", (128, 16), F32)
    Pg_d = di("Pg", (128, 32), F32)
    Pb_d = di("Pb", (32, 128), F32)
    ident_d = di("ident", (128, 128), BF)
    out_d = nc.dram_tensor("out", [T * B, VS], BF, kind="ExternalOutput")
    if dbg:
        dbg_hist = nc.dram_tensor("dbg_hist", [128, 8 * 256], BF, kind="ExternalOutput")
        dbg_ctxh = nc.dram_tensor("dbg_ctxh", [128, 8 * 128], BF, kind="ExternalOutput")
        dbg_cT = nc.dram_tensor("dbg_cT", [128, 256], F32, kind="ExternalOutput")

    AF = mybir.ActivationFunctionType
    AX = mybir.AxisListType
    ALU = mybir.AluOpType
    with tile.TileContext(nc) as tc:
        with tc.tile_pool(name="cst", bufs=1) as cst, \
             tc.tile_pool(name="wk", bufs=3) as wk, \
             tc.tile_pool(name="wkd", bufs=3) as wkd, \
             tc.tile_pool(name="wkv", bufs=2) as wkv, \
             tc.tile_pool(name="psg", bufs=2, space="PSUM") as psg, \
             tc.tile_pool(name="psd", bufs=2, space="PSUM") as psd, \
             tc.tile_pool(name="psps", bufs=1, space="PSUM") as psps, \
             tc.tile_pool(name="psm", bufs=1, space="PSUM") as psm:
            # ---- persistent SBUF ----
            hist = cst.tile([128, 8 * 256], BF, name="hist")
            cT = cst.tile([128, 256], F32, name="cT")
            # h0/c0 land before the weight loads so step 0 starts immediately
            nc.sync.dma_start(hist[:, 7 * 256:8 * 256], h0_d[:])
            nc.sync.dma_start(cT[:], c0_d[:])
            iwT = _load_tiled(nc, cst, iwT_d, 4, 640, BF, "iwT")
            Wa = _load_tiled(nc, cst, Wa_d, 8, 512, BF, "Wa")
            featsT = _load_tiled(nc, cst, featsT_d, 4, 2048, BF, "featsT")
            fblk = _load_tiled(nc, cst, fblk_d, 16, 512, BF, "fblk")
            Wc2o = _load_tiled(nc, cst, Wc2o_d, 4, 512, BF, "Wc2o")
            Wh2o = _load_tiled(nc, cst, Wh2o_d, 8, 512, BF, "Wh2o")
            F8 = mybir.dt.float8e3
            Wi1 = _load_tiled(nc, cst, Wi1_d, 4, 4096, F8, "Wi1")
            Wi2 = _load_tiled(nc, cst, Wi2_d, 4, 4096, F8, "Wi2")
            Whh = _load_tiled(nc, cst, Whh_d, 8, 4096, F8, "Whh")
            small = [("biasLhs", biasLhs_d, [32, 128], BF),
                     ("indic32", indic32_d, [32, 1024], BF),
                     ("bdec", bdec_d, [128, 4], F32),
                     ("maskE", maskE_d, [128, 512], BF),
                     ("padT", padT_d, [128, 16], F32),
                     ("Pg", Pg_d, [128, 32], F32),
                     ("Pb", Pb_d, [32, 128], F32),
                     ("ident", ident_d, [128, 128], BF)]
            sm = {}
            for nm, d, sh, dt in small:
                sm[nm] = cst.tile(sh, dt, name=nm)
                nc.sync.dma_start(sm[nm][:], d[:])
            biasLhs, indic32, bdec = sm["biasLhs"], sm["indic32"], sm["bdec"]
            maskE, padT, Pg, Pb, ident = (sm["maskE"], sm["padT"], sm["Pg"],
                                          sm["Pb"], sm["ident"])
            ctxh = cst.tile([128, 8 * 128], BF, name="ctxh")
            # decT ring of 2 groups: col kt*256 + (g%2)*128 + (t%4)*32 + b
            decT = cst.tile([128, 4 * 256], BF, name="decT")
            mm = nc.tensor.matmul

            def gates_iw(t):
                _sec(nc, 'gatesiw')
                """allocate gps for step t; bias + iw part (no h/ctx dep)"""
                gps = psg.tile([128, 1024], F32, name="gps")
                mm(gps[:, 0:512], biasLhs[:], indic32[:, 0:512],
                   start=True, stop=False)
                mm(gps[:, 512:1024], biasLhs[:], indic32[:, 512:1024],
                   start=True, stop=False)
                for bi in range(32):
                    blk = gps[:, bi * 32:(bi + 1) * 32]
                    for kt in range(4):
                        mm(blk, Wi2[:, kt * 4096 + bi * 128: kt * 4096 + (bi + 1) * 128],
                           iwT[:, kt * 640 + t * 32: kt * 640 + (t + 1) * 32],
                           start=False, stop=False)
                return gps

            def dec_group(gq):
                _sec(nc, 'dec')
                # Wh2o/Wc2o lhsT tiles streamed from HBM per group
                s0 = (4 * gq) % 8
                g2 = gq % 2
                hv = hist[:].rearrange("p (s r b) -> p s r b", s=8, b=32)
                cv = ctxh[:].rearrange("p (s r b) -> p s r b", s=8, b=32)
                dps = psd.tile([128, 512], F32, name="dps", tag="dv")
                for r in range(8):
                    for m in range(4):
                        mm(dps[:, m * 128:(m + 1) * 128],
                           Wh2o[:, r * 512 + m * 128: r * 512 + (m + 1) * 128],
                           hv[:, s0:s0 + 4, r, :],
                           start=(r == 0 and m == 0), stop=False)
                for r in range(4):
                    for m in range(4):
                        mm(dps[:, m * 128:(m + 1) * 128],
                           Wc2o[:, r * 512 + m * 128: r * 512 + (m + 1) * 128],
                           cv[:, s0:s0 + 4, r, :], start=False, stop=False)
                for m in range(4):
                    mm(dps[:, m * 128:(m + 1) * 128], ident[:],
                       iwT[:, m * 640 + gq * 128: m * 640 + (gq + 1) * 128],
                       start=False, stop=True)
                for m in range(4):
                    nc.scalar.activation(
                        decT[:, m * 256 + g2 * 128: m * 256 + (g2 + 1) * 128],
                        dps[:, m * 128:(m + 1) * 128], AF.Tanh,
                        bias=bdec[:, m:m + 1])

            def vocab_chunks(gq, cis):
                # consecutive chunks paired into one output DMA so the store
                # has >=512B contiguous runs (sub-512B runs pay 2x latency)
                _sec(nc, 'vocab')
                g2 = gq % 2
                cis = list(cis)
                i = 0
                while i < len(cis):
                    pair = cis[i:i + 2]
                    if len(pair) == 2 and pair[1] != pair[0] + 1:
                        pair = pair[:1]
                    lgv = wkv.tile([128, 2 * CW], BF, name="lgv")
                    for j, ci in enumerate(pair):
                        wt = wkv.tile([128, 4 * CW], BF, name="wt", bufs=8)
                        nc.sync.dma_start(
                            wt[:], Wout_d[:, ci * 4 * CW:(ci + 1) * 4 * CW])
                        vps = psd.tile([128, CW], F32, name="vps", tag="dv")
                        for kt in range(4):
                            mm(vps[:],
                               decT[:, kt * 256 + g2 * 128: kt * 256 + (g2 + 1) * 128],
                               wt[:, kt * CW:(kt + 1) * CW],
                               start=(kt == 0), stop=(kt == 3))
                        nc.scalar.copy(lgv[:, j * CW:(j + 1) * CW], vps[:])
                    nc.sync.dma_start(
                        out_d[gq * 128:(gq + 1) * 128,
                              pair[0] * CW: pair[0] * CW + len(pair) * CW],
                        lgv[:, 0:len(pair) * CW])
                    i += len(pair)

            for rep in range(repeat):
                if rep > 0:
                    nc.sync.dma_start(hist[:, 7 * 256:8 * 256], h0_d[:])
                    nc.sync.dma_start(cT[:], c0_d[:])
                gps = None
                for t in range(T):
                    s = (t - 1) % 8
                    w = t % 8
                    hs = lambda kt: hist[:, s * 256 + kt * 32: s * 256 + kt * 32 + 32]
                    # -- u = Wa^T h  (512, 32) as (128, 4x32)
                    _sec(nc, 'u')
                    pu = psm.tile([128, 128], F32, name="pu", tag="x")
                    for m in range(4):
                        for r in range(8):
                            mm(pu[:, m * 32:(m + 1) * 32],
                               Wa[:, r * 512 + m * 128: r * 512 + (m + 1) * 128],
                               hs(r), start=(m == 0 and r == 0), stop=(r == 7))
                    u = wk.tile([128, 128], BF, name="u")
                    nc.scalar.copy(u[:], pu[:])
                    # -- scoresT (2048, 32) as (128, 16x32)
                    _sec(nc, 'scoresT')
                    sps = psps.tile([128, 512], F32, name="sps")
                    for m in range(16):
                        for kt in range(4):
                            mm(sps[:, m * 32:(m + 1) * 32],
                               featsT[:, kt * 2048 + m * 128: kt * 2048 + (m + 1) * 128],
                               u[:, kt * 32:(kt + 1) * 32],
                               start=(m == 0 and kt == 0), stop=(kt == 3))
                    # -- gates bias+iw part (t=0 only; later steps emit it
                    # at the end of the previous step to fill the LSTM gap)
                    _sec(nc, 'gates0')
                    if gps is None:
                        gps = gates_iw(0)
                    _sec(nc, 'gatesWhh')
                    # -- gates h part
                    for bi in range(32):
                        blk = gps[:, bi * 32:(bi + 1) * 32]
                        for kt in range(8):
                            mm(blk, Whh[:, kt * 4096 + bi * 128: kt * 4096 + (bi + 1) * 128],
                               hs(kt), start=False, stop=False)
                    _sec(nc, 'dec+v2')
                    # deferred dec for the previous group
                    if t % 4 == 0 and t > 0:
                        dec_group(t // 4 - 1)
                    # 2 vocab chunks fill the softmax window
                    if t >= 4:
                        _lo, _hi = [(0, 1), (1, 6), (6, 11), (11, 16)][t % 4]
                        _mid = _lo + (1 if t % 4 == 0 else 2)
                        vocab_chunks(t // 4 - 1, range(_lo, _mid))
                    # -- diag extract + softmax (no max-sub; |scores| < 88)
                    _sec(nc, 'softmax')
                    nc.vector.tensor_mul(sps[:], sps[:], maskE[:])
                    sd = wk.tile([128, 16], F32, name="sd")
                    nc.vector.reduce_sum(
                        sd[:], sps[:].rearrange("p (m c) -> p m c", c=32), axis=AX.X)
                    nc.vector.tensor_add(sd[:], sd[:], padT[:])
                    ex = wk.tile([128, 16], BF, name="ex")
                    rows = wk.tile([128, 1], F32, name="rows")
                    nc.scalar.activation(ex[:], sd[:], AF.Exp, accum_out=rows[:])
                    # dummy: pulls the sigmoid-set table load (1.3us) into the
                    # post-exp window instead of the LSTM critical chain
                    dum = wk.tile([128, 1], F32, name="dum")
                    nc.scalar.activation(dum[:], rows[:], AF.Sigmoid)
                    pS = psm.tile([32, 1], F32, name="pS", tag="x")
                    mm(pS[:], Pg[:], rows[:], start=True, stop=True)
                    rS = wk.tile([32, 1], F32, name="rS")
                    nc.vector.reciprocal(rS[:], pS[:])
                    rb = psm.tile([128, 1], F32, name="rb", tag="x")
                    mm(rb[:], Pb[:], rS[:], start=True, stop=True)
                    # fused normalize + diag expansion: aEs = (ex * rb) * maskE
                    aEs = wk.tile([128, 512], BF, name="aEs")
                    nc.vector.scalar_tensor_tensor(
                        aEs[:].rearrange("p (m c) -> p m c", c=32),
                        ex[:].rearrange("p m -> p m ()").broadcast_to([128, 16, 32]),
                        rb[:],
                        maskE[:].rearrange("p (m c) -> p m c", c=32),
                        ALU.mult, ALU.mult)
                    # -- ctxT (512, 32) as (128, 4x32)
                    _sec(nc, 'ctx')
                    cps = psm.tile([128, 128], F32, name="cps", tag="x")
                    for m in range(4):
                        for r in range(16):
                            mm(cps[:, m * 32:(m + 1) * 32],
                               fblk[:, r * 512 + m * 128: r * 512 + (m + 1) * 128],
                               aEs[:, r * 32:(r + 1) * 32],
                               start=(m == 0 and r == 0), stop=(r == 15))
                    nc.scalar.copy(ctxh[:, w * 128:(w + 1) * 128], cps[:])
                    # -- gates ctx part: bank0 gates (g,f) first, their
                    # activations + c-mul overlap the bank1 (i,o) matmuls
                    # (bank-aligned so no PSUM-bank ping-pong)
                    _sec(nc, 'Wi1+act')
                    gG, gF, gI, gO = (gps[:, 0:256], gps[:, 256:512],
                                      gps[:, 512:768], gps[:, 768:1024])
                    tGs = wk.tile([128, 256], BF, name="tGs", bufs=2)
                    def wi1_sec(gsec):
                        for bi in range(gsec * 8, gsec * 8 + 8):
                            blk = gps[:, bi * 32:(bi + 1) * 32]
                            for kt in range(4):
                                mm(blk, Wi1[:, kt * 4096 + bi * 128: kt * 4096 + (bi + 1) * 128],
                                   ctxh[:, w * 128 + kt * 32: w * 128 + (kt + 1) * 32],
                                   start=False, stop=(kt == 3))
                    wi1_sec(0)
                    wi1_sec(1)
                    nc.scalar.activation(tGs[:], gG, AF.Tanh, scale=1.0 / WSCALE)
                    sF = wk.tile([128, 256], BF, name="sF", bufs=2)
                    nc.scalar.activation(sF[:], gF, AF.Sigmoid, scale=1.0 / WSCALE)
                    nc.vector.tensor_mul(cT[:], cT[:], sF[:])
                    wi1_sec(2)
                    wi1_sec(3)
                    nc.scalar.activation(gI, gI, AF.Sigmoid, scale=1.0 / WSCALE)
                    # -- p-state heater: dep-free dummy matmuls pinned behind
                    # sF keep the PE clock streak alive through the LSTM tail
                    # (a broken streak costs ~3us of half-speed ramp-up)
                    heat = psm.tile([128, 32], F32, name="heat", tag="x")
                    for _hk in range(112):
                        mm(heat[:], Wa[:, 0:128], sF[:, 0:32],
                           start=True, stop=True)
                    # -- LSTM-gap fillers: next step's dep-free gate matmuls
                    # first (no DMA dependency), then 2 more vocab chunks
                    _sec(nc, 'giw+v2b')
                    gps_next = gates_iw(t + 1) if t + 1 < T else None
                    if t >= 4:
                        vocab_chunks(t // 4 - 1, range(_mid, _hi))
                    # -- LSTM elementwise tail (activations emitted above)
                    _sec(nc, 'lstmtail')
                    nc.vector.tensor_mul(gI, gI, tGs[:])
                    nc.scalar.activation(gO, gO, AF.Sigmoid, scale=1.0 / WSCALE)
                    nc.vector.tensor_add(cT[:], cT[:], gI)
                    tCs = wk.tile([128, 256], BF, name="tCs", bufs=2)
                    nc.scalar.activation(tCs[:, 0:128], cT[:, 0:128], AF.Tanh)
                    nc.vector.tensor_mul(hist[:, w * 256: w * 256 + 128],
                                         gps[:, 768:896], tCs[:, 0:128])
                    nc.scalar.activation(tCs[:, 128:256], cT[:, 128:256], AF.Tanh)
                    nc.vector.tensor_mul(hist[:, w * 256 + 128:(w + 1) * 256],
                                         gps[:, 896:1024], tCs[:, 128:256])
                    gps = gps_next
                # tail: last dec group + its vocab
                dec_group(4)
                vocab_chunks(4, range(16))
                if dbg:
                    nc.sync.dma_start(dbg_hist[:], hist[:])
                    nc.sync.dma_start(dbg_ctxh[:], ctxh[:])
                    nc.sync.dma_start(dbg_cT[:], cT[:])
    nc.finalize()
    return nc


def kernel(**inputs) -> np.ndarray:
    if "nc" not in _BUILT:
        _BUILT["nc"] = build()
    nc = _BUILT["nc"]
    in_maps = host_prep(inputs)
    res = run_bass_kernel_spmd(nc, in_maps, core_ids=list(range(NC)))
    full = np.concatenate(
        [np.asarray(res.results[k]["out"]) for k in range(NC)], axis=1)
    # (640, 32000) bf16, row t*32+b -> (B, T, V) f32
    out = np.ascontiguousarray(
        full.reshape(T, B, V).transpose(1, 0, 2)).astype(np.float32)
    b_out = np.asarray(inputs["b_out"], np.float32)
    if np.any(b_out):
        out += b_out[None, None, :]
    return out



# revision 3
# speedup vs baseline: 1.0273x; 1.0273x over previous
"""Trainium2 Bass kernel for nn_AttnDecoderRNN (B=32,T=20,L=49,F=512,H=1024,V=32000).

Zero-collective design across 8 NeuronCores:
- The attention-LSTM recurrence is fully REPLICATED on every core (tensor-
  parallel splits need a per-step AllGather of h, which dominates cost).
- Only the vocab projection is tensor-parallel: core k owns W_out rows
  [4000k, 4000(k+1)) and emits a (640, 4000) bf16 logit shard; the host
  concatenation is the all-gather. b_out is added host-side (skipped if 0).

Per-core schedule (all layouts feature-on-partition):
- scoresT attention: sps[(l,b), b'] = feats . (Wa^T h), diagonal extracted
  with a one-hot mask + grouped reduce; softmax runs in the (l%4 x b, l//4)
  layout with tiny PE sum/broadcast matmuls; normalize+expand fused into one
  scalar_tensor_tensor.
- gates: one PSUM accumulation (bias matmul + iw + h + ctx parts); the
  LSTM elementwise runs mostly in place on PSUM; a dummy sigmoid after the
  softmax Exp prefetches the ACT table set off the critical chain.
- dec groups (4 steps) with Wh2o/Wc2o streamed from HBM; vocab chunks are
  interleaved into the step loop (W_out streamed) to keep the PE fed
  through the softmax/LSTM dependency-chain windows.
"""
import sys

sys.path.insert(0, "/opt/trn_rl_repo")
import numpy as np
import ml_dtypes

import concourse.bass as bass
import concourse.mybir as mybir
import concourse.tile as tile
from concourse import bacc
from concourse.bass_utils import run_bass_kernel_spmd

B, T, L, F, H, V = 32, 20, 49, 512, 1024, 32000
LP = 64
NC = 8
VS = V // NC      # 4000
CW = 250          # vocab chunk width (16 chunks per group)
BF = mybir.dt.bfloat16
F32 = mybir.dt.float32
NBF = ml_dtypes.bfloat16
NF8 = ml_dtypes.float8_e3m4
WSCALE = 128.0
GPERM = [2, 1, 0, 3]   # gate storage g,f,i,o: bank0={g,f} early, bank1={i,o} late

_BUILT = {}
SECTIONS = []


def _sec(nc, label):
    SECTIONS.append((int(nc.get_next_instruction_name().split('-')[1]), label))


def host_prep(inputs):
    f32 = lambda x: np.asarray(x, np.float32)
    feats = f32(inputs["features"])                    # (B, F, L)
    cap = np.asarray(inputs["captions"])
    emb = np.asarray(inputs["embed_table"])
    fpad = np.zeros((LP, B, F), np.float32)
    fpad[:L] = feats.transpose(2, 0, 1)
    fblk = fpad.reshape(LP * B, F)                     # (2048, 512)
    featsT = np.ascontiguousarray(fblk.T)              # (512, 2048) col l*32+b
    h0 = np.tanh(feats.mean(axis=2) @ f32(inputs["W_init"]).T + f32(inputs["b_init"]))
    h0T = h0.T                                         # (1024, 32)
    h0slot = np.ascontiguousarray(
        h0T.reshape(8, 128, B).transpose(1, 0, 2)).reshape(128, 256)
    e = f32(emb[cap])
    iw = np.concatenate([np.zeros((B, 1, F), np.float32), e[:, :-1]], axis=1)
    iwT = np.ascontiguousarray(iw.transpose(2, 1, 0)).reshape(F, T * B)
    Wih = f32(inputs["W_ih"])
    # permute gate blocks: storage pos p holds original gate GPERM[p]
    def gperm_cols(W):                                 # W (K, 4096) -> permuted
        return np.ascontiguousarray(
            W.reshape(-1, 4, 1024)[:, GPERM].reshape(-1, 4096))
    Whh = gperm_cols(f32(inputs["W_hh"]).T)            # (1024, 4096)
    Wi1 = gperm_cols(Wih[:, :F].T)                     # (512, 4096)
    Wi2 = gperm_cols(Wih[:, F:].T)                     # (512, 4096)
    bg = (f32(inputs["b_ih"]) + f32(inputs["b_hh"])).reshape(4, 8, 128)[GPERM]
    indic32 = np.zeros((32, 1024), np.float32)
    for k in range(32):
        indic32[k, k * 32:(k + 1) * 32] = 1.0
    maskE = np.zeros((128, 512), np.float32)
    for p in range(128):
        maskE[p, np.arange(16) * 32 + (p % 32)] = 1.0
    padT = np.zeros((128, 16), np.float32)
    for p in range(128):
        for m in range(16):
            if m * 4 + p // 32 >= L:
                padT[p, m] = -30000.0
    Pg = np.zeros((128, 32), np.float32)
    for p in range(128):
        Pg[p, p % 32] = 1.0
    Pb = np.ascontiguousarray(Pg.T)
    bdec = (f32(inputs["b_h2o"]) + f32(inputs["b_c2o"])).reshape(4, 128).T
    shared = {
        "featsT": featsT.astype(NBF), "fblk": fblk.astype(NBF),
        "h0slot": h0slot.astype(NBF), "c0": h0slot.astype(np.float32),
        "iwT": iwT.astype(NBF),
        "Wa": f32(inputs["Wa"]).astype(NBF),           # (1024, 512) lhsT
        # gate weights in float8-e3m4 at x128 pre-scale (halves the
        # startup-critical load bytes); compensated by scale=1/128 on the
        # gate activations. x128 lifts the 0.02-scale weights into e3m4's
        # normal range (values +-13 << 31 max).
        "Whh": (Whh * WSCALE).astype(NF8),
        "Wi1": (Wi1 * WSCALE).astype(NF8),
        "Wi2": (Wi2 * WSCALE).astype(NF8),
        "biasLhs": (bg.reshape(32, 128) * WSCALE).astype(NBF),
        "indic32": indic32.astype(NBF),
        "Wh2o": f32(inputs["W_h2o"]).T.astype(NBF),    # (1024, 512)
        "Wc2o": f32(inputs["W_c2o"]).T.astype(NBF),    # (512, 512)
        "bdec": np.ascontiguousarray(bdec),
        "maskE": maskE.astype(NBF), "padT": padT,
        "Pg": Pg, "Pb": Pb,
        "ident": np.eye(128, dtype=np.float32).astype(NBF),
    }
    WoutT = f32(inputs["W_out"]).T                     # (512, 32000)
    in_maps = []
    for k in range(NC):
        m = dict(shared)
        # partition-major chunk layout: row p holds [ci][kt][n] so each
        # chunk DMA is one contiguous (128, 1000-elem) slice (>=512B runs)
        m["Wout"] = np.ascontiguousarray(
            WoutT[:, VS * k:VS * (k + 1)].reshape(4, 128, 16, CW)
            .transpose(1, 2, 0, 3).reshape(128, 64 * CW)).astype(NBF)
        in_maps.append(m)
    return in_maps


def _load_tiled(nc, pool, dram, KT, N, dtype, name):
    """dram (KT*128, N) -> sbuf (128, KT*N), col block kt holds rows kt*128.."""
    t = pool.tile([128, KT * N], dtype, name=name)
    src = dram[:].rearrange("(r p) n -> p r n", p=128)
    dst = t[:].rearrange("p (r n) -> p r n", n=N)
    nc.sync.dma_start(dst, src)
    return t


def build(repeat=1, dbg=False):
    nc = bacc.Bacc("TRN2", target_bir_lowering=False, debug=False, num_devices=NC)
    di = lambda nm, sh, dt: nc.dram_tensor(nm, list(sh), dt, kind="ExternalInput")
    featsT_d = di("featsT", (512, 2048), BF)
    fblk_d = di("fblk", (2048, 512), BF)
    h0_d = di("h0slot", (128, 256), BF)
    c0_d = di("c0", (128, 256), F32)
    iwT_d = di("iwT", (512, 640), BF)
    Wa_d = di("Wa", (1024, 512), BF)
    F8 = mybir.dt.float8e3
    Whh_d = di("Whh", (1024, 4096), F8)
    Wi1_d = di("Wi1", (512, 4096), F8)
    Wi2_d = di("Wi2", (512, 4096), F8)
    biasLhs_d = di("biasLhs", (32, 128), BF)
    indic32_d = di("indic32", (32, 1024), BF)
    Wh2o_d = di("Wh2o", (1024, 512), BF)
    Wc2o_d = di("Wc2o", (512, 512), BF)
    bdec_d = di("bdec", (128, 4), F32)
    Wout_d = di("Wout", (128, 64 * CW), BF)
    maskE_d = di("maskE", (128, 512), BF)
    padT_d = di("padT", (128, 16), F32)
    Pg_d = di("Pg", (128, 32), F32)
    Pb_d = di("Pb", (32, 128), F32)
    ident_d = di("ident", (128, 128), BF)
    out_d = nc.dram_tensor("out", [T * B, VS], BF, kind="ExternalOutput")
    if dbg:
        dbg_hist = nc.dram_tensor("dbg_hist", [128, 8 * 256], BF, kind="ExternalOutput")
        dbg_ctxh = nc.dram_tensor("dbg_ctxh", [128, 8 * 128], BF, kind="ExternalOutput")
        dbg_cT = nc.dram_tensor("dbg_cT", [128, 256], F32, kind="ExternalOutput")

    AF = mybir.ActivationFunctionType
    AX = mybir.AxisListType
    ALU = mybir.AluOpType
    with tile.TileContext(nc) as tc:
        with tc.tile_pool(name="cst", bufs=1) as cst, \
             tc.tile_pool(name="wk", bufs=3) as wk, \
             tc.tile_pool(name="wkd", bufs=3) as wkd, \
             tc.tile_pool(name="wkv", bufs=2) as wkv, \
             tc.tile_pool(name="psg", bufs=2, space="PSUM") as psg, \
             tc.tile_pool(name="psd", bufs=2, space="PSUM") as psd, \
             tc.tile_pool(name="psps", bufs=1, space="PSUM") as psps, \
             tc.tile_pool(name="psm", bufs=1, space="PSUM") as psm:
            # ---- persistent SBUF ----
            hist = cst.tile([128, 8 * 256], BF, name="hist")
            cT = cst.tile([128, 256], F32, name="cT")
            # h0/c0 land before the weight loads so step 0 starts immediately
            nc.sync.dma_start(hist[:, 7 * 256:8 * 256], h0_d[:])
            nc.sync.dma_start(cT[:], c0_d[:])
            # load order = step-0 consumption order: u(Wa) -> scoresT(featsT)
            # -> softmax consts -> gates_iw(iwT,Wi2) -> gatesWhh(Whh) ->
            # ctx(fblk) -> Wi1 -> dec(Wh2o,Wc2o, needed t>=4) -> Wout (t>=4)
            Wa = _load_tiled(nc, cst, Wa_d, 8, 512, BF, "Wa")
            featsT = _load_tiled(nc, cst, featsT_d, 4, 2048, BF, "featsT")
            small = [("biasLhs", biasLhs_d, [32, 128], BF),
                     ("indic32", indic32_d, [32, 1024], BF),
                     ("bdec", bdec_d, [128, 4], F32),
                     ("maskE", maskE_d, [128, 512], BF),
                     ("padT", padT_d, [128, 16], F32),
                     ("Pg", Pg_d, [128, 32], F32),
                     ("Pb", Pb_d, [32, 128], F32),
                     ("ident", ident_d, [128, 128], BF)]
            sm = {}
            for nm, d, sh, dt in small:
                sm[nm] = cst.tile(sh, dt, name=nm)
                nc.sync.dma_start(sm[nm][:], d[:])
            iwT = _load_tiled(nc, cst, iwT_d, 4, 640, BF, "iwT")
            F8 = mybir.dt.float8e3
            Wi2 = _load_tiled(nc, cst, Wi2_d, 4, 4096, F8, "Wi2")
            Whh = _load_tiled(nc, cst, Whh_d, 8, 4096, F8, "Whh")
            fblk = _load_tiled(nc, cst, fblk_d, 16, 512, BF, "fblk")
            Wi1 = _load_tiled(nc, cst, Wi1_d, 4, 4096, F8, "Wi1")
            Wh2o = _load_tiled(nc, cst, Wh2o_d, 8, 512, BF, "Wh2o")
            Wc2o = _load_tiled(nc, cst, Wc2o_d, 4, 512, BF, "Wc2o")
            WoutS = cst.tile([128, 64 * CW], BF, name="WoutS")
            nc.sync.dma_start(WoutS[:], Wout_d[:])
            biasLhs, indic32, bdec = sm["biasLhs"], sm["indic32"], sm["bdec"]
            maskE, padT, Pg, Pb, ident = (sm["maskE"], sm["padT"], sm["Pg"],
                                          sm["Pb"], sm["ident"])
            ctxh = cst.tile([128, 8 * 128], BF, name="ctxh")
            # decT ring of 2 groups: col kt*256 + (g%2)*128 + (t%4)*32 + b
            decT = cst.tile([128, 4 * 256], BF, name="decT")
            mm = nc.tensor.matmul

            def gates_iw(t):
                _sec(nc, 'gatesiw')
                """allocate gps for step t; bias + iw part (no h/ctx dep)"""
                gps = psg.tile([128, 1024], F32, name="gps")
                mm(gps[:, 0:512], biasLhs[:], indic32[:, 0:512],
                   start=True, stop=False)
                mm(gps[:, 512:1024], biasLhs[:], indic32[:, 512:1024],
                   start=True, stop=False)
                for bi in range(32):
                    blk = gps[:, bi * 32:(bi + 1) * 32]
                    for kt in range(4):
                        mm(blk, Wi2[:, kt * 4096 + bi * 128: kt * 4096 + (bi + 1) * 128],
                           iwT[:, kt * 640 + t * 32: kt * 640 + (t + 1) * 32],
                           start=False, stop=False)
                return gps

            def dec_group(gq):
                _sec(nc, 'dec')
                # Wh2o/Wc2o lhsT tiles streamed from HBM per group
                s0 = (4 * gq) % 8
                g2 = gq % 2
                hv = hist[:].rearrange("p (s r b) -> p s r b", s=8, b=32)
                cv = ctxh[:].rearrange("p (s r b) -> p s r b", s=8, b=32)
                dps = psd.tile([128, 512], F32, name="dps", tag="dv")
                for r in range(8):
                    for m in range(4):
                        mm(dps[:, m * 128:(m + 1) * 128],
                           Wh2o[:, r * 512 + m * 128: r * 512 + (m + 1) * 128],
                           hv[:, s0:s0 + 4, r, :],
                           start=(r == 0 and m == 0), stop=False)
                for r in range(4):
                    for m in range(4):
                        mm(dps[:, m * 128:(m + 1) * 128],
                           Wc2o[:, r * 512 + m * 128: r * 512 + (m + 1) * 128],
                           cv[:, s0:s0 + 4, r, :], start=False, stop=False)
                for m in range(4):
                    mm(dps[:, m * 128:(m + 1) * 128], ident[:],
                       iwT[:, m * 640 + gq * 128: m * 640 + (gq + 1) * 128],
                       start=False, stop=True)
                for m in range(4):
                    nc.scalar.activation(
                        decT[:, m * 256 + g2 * 128: m * 256 + (g2 + 1) * 128],
                        dps[:, m * 128:(m + 1) * 128], AF.Tanh,
                        bias=bdec[:, m:m + 1])

            def vocab_chunks(gq, cis):
                # consecutive chunks paired into one output DMA so the store
                # has >=512B contiguous runs (sub-512B runs pay 2x latency)
                _sec(nc, 'vocab')
                g2 = gq % 2
                cis = list(cis)
                i = 0
                while i < len(cis):
                    pair = cis[i:i + 2]
                    if len(pair) == 2 and pair[1] != pair[0] + 1:
                        pair = pair[:1]
                    lgv = wkv.tile([128, 2 * CW], BF, name="lgv")
                    for j, ci in enumerate(pair):
                        vps = psd.tile([128, CW], F32, name="vps", tag="dv")
                        for kt in range(4):
                            mm(vps[:],
                               decT[:, kt * 256 + g2 * 128: kt * 256 + (g2 + 1) * 128],
                               WoutS[:, ci * 4 * CW + kt * CW: ci * 4 * CW + (kt + 1) * CW],
                               start=(kt == 0), stop=(kt == 3))
                        nc.vector.tensor_copy(lgv[:, j * CW:(j + 1) * CW], vps[:])
                    nc.sync.dma_start(
                        out_d[gq * 128:(gq + 1) * 128,
                              pair[0] * CW: pair[0] * CW + len(pair) * CW],
                        lgv[:, 0:len(pair) * CW])
                    i += len(pair)

            for rep in range(repeat):
                if rep > 0:
                    nc.sync.dma_start(hist[:, 7 * 256:8 * 256], h0_d[:])
                    nc.sync.dma_start(cT[:], c0_d[:])
                gps = None
                for t in range(T):
                    s = (t - 1) % 8
                    w = t % 8
                    hs = lambda kt: hist[:, s * 256 + kt * 32: s * 256 + kt * 32 + 32]
                    # -- u = Wa^T h  (512, 32) as (128, 4x32)
                    _sec(nc, 'u')
                    pu = psm.tile([128, 128], F32, name="pu", tag="x")
                    for m in range(4):
                        for r in range(8):
                            mm(pu[:, m * 32:(m + 1) * 32],
                               Wa[:, r * 512 + m * 128: r * 512 + (m + 1) * 128],
                               hs(r), start=(m == 0 and r == 0), stop=(r == 7))
                    u = wk.tile([128, 128], BF, name="u")
                    nc.scalar.copy(u[:], pu[:])
                    # -- scoresT (2048, 32) as (128, 16x32)
                    _sec(nc, 'scoresT')
                    sps = psps.tile([128, 512], F32, name="sps")
                    for m in range(16):
                        for kt in range(4):
                            mm(sps[:, m * 32:(m + 1) * 32],
                               featsT[:, kt * 2048 + m * 128: kt * 2048 + (m + 1) * 128],
                               u[:, kt * 32:(kt + 1) * 32],
                               start=(m == 0 and kt == 0), stop=(kt == 3))
                    # -- gates bias+iw part (t=0 only; later steps emit it
                    # at the end of the previous step to fill the LSTM gap)
                    _sec(nc, 'gates0')
                    if gps is None:
                        gps = gates_iw(0)
                    _sec(nc, 'gatesWhh')
                    # -- gates h part
                    for bi in range(32):
                        blk = gps[:, bi * 32:(bi + 1) * 32]
                        for kt in range(8):
                            mm(blk, Whh[:, kt * 4096 + bi * 128: kt * 4096 + (bi + 1) * 128],
                               hs(kt), start=False, stop=False)
                    _sec(nc, 'dec+v2')
                    # deferred dec for the previous group
                    if t % 4 == 0 and t > 0:
                        dec_group(t // 4 - 1)
                    # 2 vocab chunks fill the softmax window
                    if t >= 4:
                        _lo, _hi = [(0, 1), (1, 6), (6, 11), (11, 16)][t % 4]
                        _mid = _lo + (1 if t % 4 == 0 else 2)
                        vocab_chunks(t // 4 - 1, range(_lo, _mid))
                    # -- diag extract + softmax (no max-sub; |scores| < 88)
                    _sec(nc, 'softmax')
                    nc.vector.tensor_mul(sps[:], sps[:], maskE[:])
                    sd = wk.tile([128, 16], F32, name="sd")
                    nc.vector.reduce_sum(
                        sd[:], sps[:].rearrange("p (m c) -> p m c", c=32), axis=AX.X)
                    nc.vector.tensor_add(sd[:], sd[:], padT[:])
                    ex = wk.tile([128, 16], BF, name="ex")
                    rows = wk.tile([128, 1], F32, name="rows")
                    nc.scalar.activation(ex[:], sd[:], AF.Exp, accum_out=rows[:])
                    # dummy: pulls the sigmoid-set table load (1.3us) into the
                    # post-exp window instead of the LSTM critical chain
                    dum = wk.tile([128, 1], F32, name="dum")
                    nc.scalar.activation(dum[:], rows[:], AF.Sigmoid)
                    pS = psm.tile([32, 1], F32, name="pS", tag="x")
                    mm(pS[:], Pg[:], rows[:], start=True, stop=True)
                    rS = wk.tile([32, 1], F32, name="rS")
                    nc.vector.reciprocal(rS[:], pS[:])
                    rb = psm.tile([128, 1], F32, name="rb", tag="x")
                    mm(rb[:], Pb[:], rS[:], start=True, stop=True)
                    # fused normalize + diag expansion: aEs = (ex * rb) * maskE
                    aEs = wk.tile([128, 512], BF, name="aEs")
                    nc.vector.scalar_tensor_tensor(
                        aEs[:].rearrange("p (m c) -> p m c", c=32),
                        ex[:].rearrange("p m -> p m ()").broadcast_to([128, 16, 32]),
                        rb[:],
                        maskE[:].rearrange("p (m c) -> p m c", c=32),
                        ALU.mult, ALU.mult)
                    # -- ctxT (512, 32) as (128, 4x32)
                    _sec(nc, 'ctx')
                    cps = psm.tile([128, 128], F32, name="cps", tag="x")
                    for m in range(4):
                        for r in range(16):
                            mm(cps[:, m * 32:(m + 1) * 32],
                               fblk[:, r * 512 + m * 128: r * 512 + (m + 1) * 128],
                               aEs[:, r * 32:(r + 1) * 32],
                               start=(m == 0 and r == 0), stop=(r == 15))
                    nc.scalar.copy(ctxh[:, w * 128:(w + 1) * 128], cps[:])
                    # -- gates ctx part: bank0 gates (g,f) first, their
                    # activations + c-mul overlap the bank1 (i,o) matmuls
                    # (bank-aligned so no PSUM-bank ping-pong)
                    _sec(nc, 'Wi1+act')
                    gG, gF, gI, gO = (gps[:, 0:256], gps[:, 256:512],
                                      gps[:, 512:768], gps[:, 768:1024])
                    tGs = wk.tile([128, 256], BF, name="tGs", bufs=2)
                    def wi1_sec(gsec):
                        for bi in range(gsec * 8, gsec * 8 + 8):
                            blk = gps[:, bi * 32:(bi + 1) * 32]
                            for kt in range(4):
                                mm(blk, Wi1[:, kt * 4096 + bi * 128: kt * 4096 + (bi + 1) * 128],
                                   ctxh[:, w * 128 + kt * 32: w * 128 + (kt + 1) * 32],
                                   start=False, stop=(kt == 3))
                    wi1_sec(0)
                    wi1_sec(1)
                    nc.scalar.activation(tGs[:], gG, AF.Tanh, scale=1.0 / WSCALE)
                    sF = wk.tile([128, 256], BF, name="sF", bufs=2)
                    nc.scalar.activation(sF[:], gF, AF.Sigmoid, scale=1.0 / WSCALE)
                    nc.vector.tensor_mul(cT[:], cT[:], sF[:])
                    wi1_sec(2)
                    wi1_sec(3)
                    nc.scalar.activation(gI, gI, AF.Sigmoid, scale=1.0 / WSCALE)
                    # -- p-state heater: dep-free dummy matmuls pinned behind
                    # sF keep the PE clock streak alive through the LSTM tail
                    # (a broken streak costs ~3us of half-speed ramp-up)
                    heat = psm.tile([128, 32], F32, name="heat", tag="x")
                    for _hk in range(112):
                        mm(heat[:], Wa[:, 0:128], sF[:, 0:32],
                           start=True, stop=True)
                    # -- LSTM-gap fillers: next step's dep-free gate matmuls
                    # first (no DMA dependency), then 2 more vocab chunks
                    _sec(nc, 'giw+v2b')
                    gps_next = gates_iw(t + 1) if t + 1 < T else None
                    if t >= 4:
                        vocab_chunks(t // 4 - 1, range(_mid, _hi))
                    # -- LSTM elementwise tail (activations emitted above)
                    _sec(nc, 'lstmtail')
                    nc.vector.tensor_mul(gI, gI, tGs[:])
                    nc.scalar.activation(gO, gO, AF.Sigmoid, scale=1.0 / WSCALE)
                    nc.vector.tensor_add(cT[:], cT[:], gI)
                    tCs = wk.tile([128, 256], BF, name="tCs", bufs=2)
                    nc.scalar.activation(tCs[:, 0:128], cT[:, 0:128], AF.Tanh)
                    nc.vector.tensor_mul(hist[:, w * 256: w * 256 + 128],
                                         gps[:, 768:896], tCs[:, 0:128])
                    nc.scalar.activation(tCs[:, 128:256], cT[:, 128:256], AF.Tanh)
                    nc.vector.tensor_mul(hist[:, w * 256 + 128:(w + 1) * 256],
                                         gps[:, 896:1024], tCs[:, 128:256])
                    gps = gps_next
                # tail: last dec group + its vocab
                dec_group(4)
                vocab_chunks(4, range(16))
                if dbg:
                    nc.sync.dma_start(dbg_hist[:], hist[:])
                    nc.sync.dma_start(dbg_ctxh[:], ctxh[:])
                    nc.sync.dma_start(dbg_cT[:], cT[:])
    nc.finalize()
    return nc


def kernel(**inputs) -> np.ndarray:
    if "nc" not in _BUILT:
        _BUILT["nc"] = build()
    nc = _BUILT["nc"]
    in_maps = host_prep(inputs)
    res = run_bass_kernel_spmd(nc, in_maps, core_ids=list(range(NC)))
    full = np.concatenate(
        [np.asarray(res.results[k]["out"]) for k in range(NC)], axis=1)
    # (640, 32000) bf16, row t*32+b -> (B, T, V) f32
    out = np.ascontiguousarray(
        full.reshape(T, B, V).transpose(1, 0, 2)).astype(np.float32)
    b_out = np.asarray(inputs["b_out"], np.float32)
    if np.any(b_out):
        out += b_out[None, None, :]
    return out

